# revision 3
# baseline (speedup 1.0000x reference)
"""Trainium2 Bass kernel: multi-adapter LoRA linear via host-side fold.

y = x @ W.T + bias + 2*(x@A_g.T)@B_g.T  ==  x @ (W + 2*B_g@A_g).T + bias

Data-parallel over batch: each of 8 cores gets one batch element and its
group's folded weight matrix W'_g = W + 2*B_g@A_g (exact fp32 fold on host,
cast to fp16). The device kernel is then a pure [2048x4096] @ [4096x4096]
fp16 matmul with a DVE bias-add fused into the PSUM evacuation.

Schedule: x.T streams chunk-wise on the sync queue while W'(ob0) streams
chunk-wise on the scalar queue; phase A interleaves the first 8 output
tiles' matmuls per k-chunk (8 PSUM banks) so the PE saturates ~3us in and
never drains. Remaining obs prefetch W one tile ahead.
"""
import sys

if "/opt/trn_rl_repo" not in sys.path:
    sys.path.insert(0, "/opt/trn_rl_repo")

import numpy as np

B, S, I, O, G, R = 8, 2048, 4096, 4096, 4, 16
OB = 512  # output free-dim tile (one PSUM bank of fp32)

_CACHE = {}


def build(s=S, i=I, o=O, repeat=1):
    import concourse.bacc as bacc
    import concourse.mybir as mybir
    import concourse.tile as tile

    f16, f32 = mybir.dt.float16, mybir.dt.float32
    kt = i // 128
    mt_n = s // 128
    nob = o // OB

    nc = bacc.Bacc("TRN2", target_bir_lowering=False, debug=False)
    xT = nc.dram_tensor("xT", [i, s], f16, kind="ExternalInput").ap()
    WT = nc.dram_tensor("WT", [i, o], f16, kind="ExternalInput").ap()
    biasr = nc.dram_tensor("biasr", [128, o], f16, kind="ExternalInput").ap()
    out = nc.dram_tensor("out", [s, o], f32, kind="ExternalOutput").ap()

    with tile.TileContext(nc) as tc:
        with (
            tc.tile_pool(name="xp", bufs=1) as xp,
            tc.tile_pool(name="wp", bufs=2) as wp,
            tc.tile_pool(name="bp", bufs=1) as bp,
            tc.tile_pool(name="op", bufs=2) as op,
            tc.tile_pool(name="pp", bufs=8, space="PSUM") as pp,
        ):
          WT3 = WT.rearrange("(k p) o -> p k o", p=128)  # [128, kt, o]

          for _rep in range(repeat):
            x_sb = xp.tile([128, kt, s], f16, name="x_sb")
            bias_sb = bp.tile([128, o], f16)

            # scalar queue: W(ob0) chunk-split so the first matmuls only wait
            # on chunk 0 (~0.7us), not the whole tile; bias follows (first
            # needed by the evacuations at ~56us)
            w0 = wp.tile([128, kt, OB], f16, tag="w")
            for k in range(kt):
                nc.scalar.dma_start(w0[:, k, :], WT3[:, k, 0:OB])
            nc.scalar.dma_start(bias_sb[:, :], biasr[:, :])
            # sync queue: x chunks in k order
            for k in range(kt):
                nc.sync.dma_start(x_sb[:, k, :], xT[k * 128:(k + 1) * 128, :])

            def mm(pt, w, mt, k, start, stop):
                nc.tensor.matmul(
                    pt[:],
                    x_sb[:, k, mt * 128:mt * 128 + 128],
                    w[:, k, :],
                    start=start,
                    stop=stop,
                )

            def evac(pt, mt, ob):
                ot = op.tile([128, OB], f32, tag="ot")
                nc.vector.tensor_tensor(
                    ot[:], pt[:], bias_sb[:, ob * OB:(ob + 1) * OB],
                    op=mybir.AluOpType.add,
                )
                nc.scalar.dma_start(
                    out[mt * 128:(mt + 1) * 128, ob * OB:(ob + 1) * OB], ot[:]
                )

            def load_w(ob):
                w = wp.tile([128, kt, OB], f16, tag="w")
                nc.sync.dma_start(w[:, :, :], WT3[:, :, ob * OB:(ob + 1) * OB])
                return w

            # phase A: per k-chunk, matmuls of the first nA tiles of ob0 --
            # each arriving x chunk enables nA matmuls, PE saturates early
            nA = min(8, mt_n)
            pts = [pp.tile([128, OB], f32, tag="pt", name=f"ptA{m}") for m in range(nA)]
            for k in range(kt):
                for m in range(nA):
                    mm(pts[m], w0, m, k, k == 0, k == kt - 1)
            w_cur = load_w(1) if nob > 1 else None
            for m in range(nA):
                evac(pts[m], m, 0)

            # rest of ob0
            for mt in range(nA, mt_n):
                pt = pp.tile([128, OB], f32, tag="pt")
                for k in range(kt):
                    mm(pt, w0, mt, k, k == 0, k == kt - 1)
                evac(pt, mt, 0)

            # remaining obs, W prefetched one ahead on the sync queue
            for ob in range(1, nob):
                w = w_cur
                w_cur = load_w(ob + 1) if ob + 1 < nob else None
                for mt in range(mt_n):
                    pt = pp.tile([128, OB], f32, tag="pt")
                    for k in range(kt):
                        mm(pt, w, mt, k, k == 0, k == kt - 1)
                    evac(pt, mt, ob)
    nc.compile()
    return nc


def prep_in_maps(data, W, bias, lora_a, lora_b):
    Wf = W.astype(np.float32)
    biasr = np.ascontiguousarray(
        np.broadcast_to(bias.astype(np.float16), (128, W.shape[0]))
    )
    WgT = {}
    for g in range(G):
        Wg = Wf + 2.0 * (lora_b[g].astype(np.float32) @ lora_a[g].astype(np.float32))
        WgT[g] = np.ascontiguousarray(Wg.T.astype(np.float16))  # [I, O]
    in_maps = []
    for b in range(data.shape[0]):
        g = b // (data.shape[0] // G)
        in_maps.append({
            "xT": np.ascontiguousarray(data[b].astype(np.float16).T),  # [I, S]
            "WT": WgT[g],
            "biasr": biasr,
        })
    return in_maps


def kernel(data, W, bias, lora_a, lora_b):
    from concourse.bass_utils import run_bass_kernel_spmd

    if "nc" not in _CACHE:
        _CACHE["nc"] = build()
    nc = _CACHE["nc"]
    in_maps = prep_in_maps(data, W, bias, lora_a, lora_b)
    res = run_bass_kernel_spmd(nc, in_maps, list(range(len(in_maps))))
    return np.stack([res.results[c]["out"] for c in range(len(in_maps))], axis=0)
